# revision 15
# baseline (speedup 1.0000x reference)

# Trainium2 Bass kernel for MinConvExpLSTMCell (v12).
#
# Math (linear-space reformulation of the reference's log-space scan):
#   y = conv3x3(x, W) + b; [f_gate, i_gate, h_tilde] = split(y)
#   diff = f_gate - i_gate = conv(x, W_f - W_i) + (b_f - b_i)
#   f = sigmoid(diff);  i = sigmoid(-diff)  (both on ACT, full precision)
#   g = min(sigmoid(y), 0.5) + relu(y),  y = h_tilde + b_h
#     (sigmoid(min(y,0)) == min(sigmoid(y),0.5) by monotonicity - exact)
#   h_t = f_t * h_{t-1} + i_t * g_t,  h_{-1} = g(h0)
#
# Sharding: 8 cores = 4 batches x 2 spatial halves (16 output rows each).
#
# Matmul: K=128 tap-pair packing - x stored twice in SBUF (partitions
# 0:63 "copy A", partitions 64:127 shifted down one image row "copy B"),
# one K=128 matmul contracts two vertically-adjacent taps at once.
# 2x column tiling gives pixel-split psum (partitions 0:63 = rows 0:8 of
# the half-image, 64:127 = rows 8:16) so post runs on 128 partitions.
# rhs free dims (steps:2, rows:8, cols:32) keep the stream contiguous;
# evacuation iterates (sp, px, st) so scan-slot writes are runs-of-2.
#
# Engines per 4-step group: ACT = f/i/sigy sigmoids + relu (all psum
# eviction); DVE = s=min(sigy,.5), g=s+r, scan; GpSimd = u=i*g, memset,
# chain copy (last segment's u on DVE, 4 scan chunks, to cut the tail).
# Input DMAs are emitted before output DMAs (FIFO queues).

import sys
import numpy as np

sys.path.insert(0, "/opt/trn_rl_repo")

import ml_dtypes
from contextlib import ExitStack

import concourse.bass as bass
import concourse.bacc as bacc
import concourse.mybir as mybir
from concourse.tile import TileContext
from concourse.bass_utils import run_bass_kernel_spmd

BF16 = ml_dtypes.bfloat16
B, T, C, H, W = 4, 64, 64, 32, 32
SEG = 8
NSEG = T // SEG            # 8
NHF = SEG // 4             # 2 four-step groups per segment
HP, WP = 18, 34            # padded shard rows/cols
RC = HP * WP               # 612
RCE = RC + WP              # 646: one extra zero row for shifted copy B
PXH = 256                  # pixels per column-strip (8 rows x 32 cols)
TS = SEG + 1               # scan slots per pixel per segment
NF = PXH * TS              # 2304 scan free size
DNS = SEG * PXH            # 2048 dense free size
NCH = 2                    # scan pixel-chunks per segment
CPX = PXH // NCH           # 128 pixels per chunk

_CACHE = {}


def _build():
    f32 = mybir.dt.float32
    bf16 = mybir.dt.bfloat16
    AF = mybir.ActivationFunctionType
    OP = mybir.AluOpType

    nc = bacc.Bacc()
    xs = nc.dram_tensor("xs", [T, C, RCE], bf16, kind="ExternalInput")
    wt = nc.dram_tensor("wt", [128, 768], bf16, kind="ExternalInput")
    cst = nc.dram_tensor("cst", [128, 3 + PXH], f32, kind="ExternalInput")
    out = nc.dram_tensor("out", [NSEG, 128, NF], bf16, kind="ExternalOutput")

    with TileContext(nc) as tc, ExitStack() as ctx:
        cpool = ctx.enter_context(tc.tile_pool(name="consts", bufs=1))
        xpool = ctx.enter_context(tc.tile_pool(name="x", bufs=3))
        pspool = ctx.enter_context(tc.tile_pool(name="ps", bufs=2, space="PSUM"))
        sigpool = ctx.enter_context(tc.tile_pool(name="sig", bufs=3))
        rpool = ctx.enter_context(tc.tile_pool(name="r", bufs=2))
        spool = ctx.enter_context(tc.tile_pool(name="s", bufs=2))
        ipool = ctx.enter_context(tc.tile_pool(name="i", bufs=2))
        upool = ctx.enter_context(tc.tile_pool(name="u", bufs=3))
        hpool = ctx.enter_context(tc.tile_pool(name="h", bufs=3))

        w_sb = cpool.tile([128, 768], bf16)
        cst_sb = cpool.tile([128, 3 + PXH], f32)
        bd2 = cst_sb[:, 0:1]
        bh2 = cst_sb[:, 1:2]
        nbd2 = cst_sb[:, 2:3]
        g0c = cst_sb[:, 3:3 + PXH]

        def emit_xt_dma(s, xt):
            xtv = xt.rearrange("p (t x) -> p t x", t=SEG)
            for q in range(NHF):
                t0, t1 = s * SEG + q * 4, s * SEG + (q + 1) * 4
                tq = slice(q * 4, (q + 1) * 4)
                nc.sync.dma_start(
                    xtv[0:64, tq],
                    xs[t0:t1, :, 0:RC].rearrange("t c x -> c t x"))
                nc.sync.dma_start(
                    xtv[64:128, tq],
                    xs[t0:t1, :, WP:WP + RC].rearrange("t c x -> c t x"))

        xt_next = xpool.tile([128, SEG * RC], bf16, name="xt")
        emit_xt_dma(0, xt_next)
        nc.sync.dma_start(w_sb[:, :], wt[:, :])
        nc.sync.dma_start(cst_sb[:, :], cst[:, :])

        h_prev = None
        for s in range(NSEG):
            xt = xt_next
            xv = xt.rearrange("p (t r c) -> p t r c", t=SEG, r=HP, c=WP)

            SIGt = sigpool.tile([128, NF], bf16, name="SIGt")
            Ut = upool.tile([128, NF], bf16, name="Ut")
            Ht = hpool.tile([128, NF], bf16, name="Ht")
            Rt = rpool.tile([128, DNS], bf16, name="Rt")
            SYt = spool.tile([128, DNS], bf16, name="SYt")
            It = ipool.tile([128, DNS], bf16, name="It")
            sigv = SIGt.rearrange("p (x t) -> p x t", t=TS)
            uv = Ut.rearrange("p (x t) -> p x t", t=TS)

            # scan-restart column: f=0 at slot 0 of every pixel
            nc.gpsimd.memset(sigv[:, :, 0], 0.0)
            # u slot0 = h_{-1} for this segment (chains segments)
            if h_prev is None:
                nc.gpsimd.tensor_copy(uv[:, :, 0], g0c)
            else:
                hv = h_prev.rearrange("p (x t) -> p x t", t=TS)
                nc.gpsimd.tensor_copy(uv[:, :, 0], hv[:, :, SEG])
            for hf in range(NHF):
                ps = pspool.tile([128, 2048], f32, name="ps")
                for kappa in range(2):           # 0 = diff, 1 = h_tilde
                    po = kappa * 1024
                    for p in range(6):
                        blk = (kappa * 6 + p) * 64
                        lhsT = w_sb[:, blk:blk + 64]
                        r0, c0 = (0, p) if p < 3 else (2, p - 3)
                        for sp in range(2):
                            t0 = hf * 4 + sp * 2
                            for cs_ in range(2):
                                rhs = xv[0:128, t0:t0 + 2,
                                         r0 + 8 * cs_:r0 + 8 * cs_ + 8,
                                         c0:c0 + 32]
                                dst = ps[cs_ * 64:(cs_ + 1) * 64,
                                         po + sp * 512:po + sp * 512 + 512]
                                nc.tensor.matmul(
                                    dst, lhsT, rhs,
                                    start=(p == 0), stop=(p == 5),
                                    tile_position=(0, cs_ * 64))

                sl = slice(hf * 1024, (hf + 1) * 1024)
                pdq = ps[:, 0:1024].rearrange("p (a s x) -> p a x s",
                                              a=2, s=2, x=PXH)
                phq = ps[:, 1024:2048].rearrange("p (a s x) -> p a x s",
                                                 a=2, s=2, x=PXH)
                # psum eviction - all on ACT:
                # f = sigmoid(diff + bd) -> scan slots (runs of 2)
                sdst = sigv[:, :, 1 + hf * 4:5 + hf * 4].rearrange(
                    "p x (a s) -> p a x s", a=2, s=2)
                nc.scalar.activation(sdst, pdq, AF.Sigmoid, bias=bd2)
                # i = sigmoid(-diff - bd) -> dense (sp, px, st) layout
                nc.scalar.activation(
                    It[:, sl], pdq, AF.Sigmoid, bias=nbd2, scale=-1.0)
                # sigy = sigmoid(ht + bh); r = relu(ht + bh)
                nc.scalar.activation(SYt[:, sl], phq, AF.Sigmoid, bias=bh2)
                nc.scalar.activation(Rt[:, sl], phq, AF.Relu, bias=bh2)
                # s = min(sigy, 0.5) in place; g = s + r; u = i * g (DVE)
                nc.vector.tensor_scalar(
                    SYt[:, sl], SYt[:, sl], 0.5, None, OP.min)
                nc.vector.tensor_tensor(
                    Rt[:, sl], SYt[:, sl], Rt[:, sl], OP.add)
                udst = uv[:, :, 1 + hf * 4:5 + hf * 4].rearrange(
                    "p x (a s) -> p a x s", a=2, s=2)
                ivq = It[:, sl].rearrange("p (a x s) -> p a x s",
                                          a=2, x=PXH, s=2)
                gvq = Rt[:, sl].rearrange("p (a x s) -> p a x s",
                                          a=2, x=PXH, s=2)
                ue = nc.vector if s == NSEG - 1 else nc.gpsimd
                ue.tensor_tensor(udst, ivq, gvq, OP.mult)

            # prefetch next segment's input before this segment's output
            # DMAs (sync queue is FIFO)
            if s + 1 < NSEG:
                xt_next = xpool.tile([128, SEG * RC], bf16, name="xt")
                emit_xt_dma(s + 1, xt_next)

            # h = scan: state = f * state + u, per-pixel chains
            nch = 4 if s == NSEG - 1 else NCH
            for k in range(nch):
                cw = PXH // nch * TS
                a, b2 = k * cw, (k + 1) * cw
                nc.vector.tensor_tensor_scan(
                    Ht[:, a:b2], SIGt[:, a:b2], Ut[:, a:b2],
                    0.0, OP.mult, OP.add)
                nc.sync.dma_start(out[s, :, a:b2], Ht[:, a:b2])
            h_prev = Ht
    nc.finalize()
    return nc


def _g0(h0):
    return np.where(h0 >= 0.0, h0 + 0.5, 1.0 / (1.0 + np.exp(-h0))).astype(np.float32)


def kernel(x, conv_w, conv_b, h0):
    x = np.asarray(x, np.float32)
    conv_w = np.asarray(conv_w, np.float32)
    conv_b = np.asarray(conv_b, np.float32)
    h0 = np.asarray(h0, np.float32)

    if "nc" not in _CACHE:
        _CACHE["nc"] = _build()
    nc = _CACHE["nc"]

    wd = conv_w[0:64] - conv_w[64:128]           # [64out, 64in, 3, 3]
    wh = conv_w[128:192]
    bd = conv_b[0:64] - conv_b[64:128]
    bh = conv_b[128:192]

    wt = np.zeros((128, 768), np.float32)
    for kappa, wk in ((0, wd), (1, wh)):
        for p in range(6):
            blk = (kappa * 6 + p) * 64
            if p < 3:
                # paired taps (0,p) on rows 0:64, (1,p) on rows 64:128
                wt[0:64, blk:blk + 64] = wk[:, :, 0, p].T
                wt[64:128, blk:blk + 64] = wk[:, :, 1, p].T
            else:
                # single tap (2, p-3); rows 64:128 stay zero
                wt[0:64, blk:blk + 64] = wk[:, :, 2, p - 3].T
    wt = wt.astype(BF16)

    x4 = x.reshape(B, T, C, H, W)
    g0f = _g0(h0)                                 # [B, C, H, W]

    bd2 = np.concatenate([bd, bd])[:, None]
    bh2 = np.concatenate([bh, bh])[:, None]

    in_maps = []
    for c in range(8):
        b, half = c // 2, c % 2
        xsh = np.zeros((T, C, HP + 1, WP), np.float32)
        if half == 0:
            xsh[:, :, 1:18, 1:33] = x4[b, :, :, 0:17, :]
        else:
            xsh[:, :, 0:17, 1:33] = x4[b, :, :, 15:32, :]
        xsh = xsh.reshape(T, C, RCE).astype(BF16)
        r16 = g0f[b, :, 16 * half:16 * half + 16, :]     # [64, 16, 32]
        g0c = np.concatenate(
            [r16[:, 0:8, :].reshape(64, PXH),
             r16[:, 8:16, :].reshape(64, PXH)], 0)       # [128, 256]
        cstc = np.concatenate(
            [bd2, bh2, -bd2, g0c], 1).astype(np.float32)
        in_maps.append({"xs": xsh, "wt": wt, "cst": cstc})

    _CACHE["in_maps"] = in_maps
    res = run_bass_kernel_spmd(nc, in_maps, core_ids=list(range(8)))

    outf = np.empty((B, T, C, H, W), np.float32)
    for c in range(8):
        b, half = c // 2, c % 2
        arr = np.asarray(res.results[c]["out"], np.float32)
        arr = arr.reshape(NSEG, 128, PXH, TS)[:, :, :, 1:]
        arr = arr.transpose(0, 3, 1, 2).reshape(T, 128, 8, 32)
        outf[b, :, :, 16 * half:16 * half + 8, :] = arr[:, 0:64]
        outf[b, :, :, 16 * half + 8:16 * half + 16, :] = arr[:, 64:128]
    return outf.reshape(B * T, C, H, W)


# revision 16
# speedup vs baseline: 1.0375x; 1.0375x over previous

# Trainium2 Bass kernel for MinConvExpLSTMCell (v12).
#
# Math (linear-space reformulation of the reference's log-space scan):
#   y = conv3x3(x, W) + b; [f_gate, i_gate, h_tilde] = split(y)
#   diff = f_gate - i_gate = conv(x, W_f - W_i) + (b_f - b_i)
#   f = sigmoid(diff);  i = sigmoid(-diff)  (both on ACT, full precision)
#   g = min(sigmoid(y), 0.5) + relu(y),  y = h_tilde + b_h
#     (sigmoid(min(y,0)) == min(sigmoid(y),0.5) by monotonicity - exact)
#   h_t = f_t * h_{t-1} + i_t * g_t,  h_{-1} = g(h0)
#
# Sharding: 8 cores = 4 batches x 2 spatial halves (16 output rows each).
#
# Matmul: K=128 tap-pair packing - x stored twice in SBUF (partitions
# 0:63 "copy A", partitions 64:127 shifted down one image row "copy B"),
# one K=128 matmul contracts two vertically-adjacent taps at once.
# 2x column tiling gives pixel-split psum (partitions 0:63 = rows 0:8 of
# the half-image, 64:127 = rows 8:16) so post runs on 128 partitions.
# rhs free dims (steps:2, rows:8, cols:32) keep the stream contiguous;
# evacuation iterates (sp, px, st) so scan-slot writes are runs-of-2.
#
# Engines per 4-step group: ACT = f/i/sigy sigmoids + relu (all psum
# eviction); DVE = s=min(sigy,.5), g=s+r, scan; GpSimd = u=i*g, memset,
# chain copy (last segment's u on DVE, 4 scan chunks, to cut the tail).
# Input DMAs are emitted before output DMAs (FIFO queues).

import sys
import numpy as np

sys.path.insert(0, "/opt/trn_rl_repo")

import ml_dtypes
from contextlib import ExitStack

import concourse.bass as bass
import concourse.bacc as bacc
import concourse.mybir as mybir
from concourse.tile import TileContext
from concourse.bass_utils import run_bass_kernel_spmd

BF16 = ml_dtypes.bfloat16
B, T, C, H, W = 4, 64, 64, 32, 32
SEG = 8
NSEG = T // SEG            # 8
NHF = SEG // 4             # 2 four-step groups per segment
HP, WP = 18, 34            # padded shard rows/cols
RC = HP * WP               # 612
RCE = RC + WP              # 646: one extra zero row for shifted copy B
PXH = 256                  # pixels per column-strip (8 rows x 32 cols)
TS = SEG + 1               # scan slots per pixel per segment
NF = PXH * TS              # 2304 scan free size
DNS = SEG * PXH            # 2048 dense free size
NCH = 2                    # scan pixel-chunks per segment
CPX = PXH // NCH           # 128 pixels per chunk

_CACHE = {}


def _build():
    f32 = mybir.dt.float32
    bf16 = mybir.dt.bfloat16
    AF = mybir.ActivationFunctionType
    OP = mybir.AluOpType

    nc = bacc.Bacc()
    xs = nc.dram_tensor("xs", [T, C, RCE], bf16, kind="ExternalInput")
    wt = nc.dram_tensor("wt", [128, 768], bf16, kind="ExternalInput")
    cst = nc.dram_tensor("cst", [128, 3 + PXH], f32, kind="ExternalInput")
    out = nc.dram_tensor("out", [NSEG, 128, NF], bf16, kind="ExternalOutput")

    with TileContext(nc) as tc, ExitStack() as ctx:
        cpool = ctx.enter_context(tc.tile_pool(name="consts", bufs=1))
        xpool = ctx.enter_context(tc.tile_pool(name="x", bufs=3))
        pspool = ctx.enter_context(tc.tile_pool(name="ps", bufs=2, space="PSUM"))
        sigpool = ctx.enter_context(tc.tile_pool(name="sig", bufs=3))
        rpool = ctx.enter_context(tc.tile_pool(name="r", bufs=2))
        spool = ctx.enter_context(tc.tile_pool(name="s", bufs=2))
        ipool = ctx.enter_context(tc.tile_pool(name="i", bufs=2))
        upool = ctx.enter_context(tc.tile_pool(name="u", bufs=3))
        hpool = ctx.enter_context(tc.tile_pool(name="h", bufs=3))

        w_sb = cpool.tile([128, 768], bf16)
        cst_sb = cpool.tile([128, 3 + PXH], f32)
        bd2 = cst_sb[:, 0:1]
        bh2 = cst_sb[:, 1:2]
        nbd2 = cst_sb[:, 2:3]
        g0c = cst_sb[:, 3:3 + PXH]

        def emit_xt_dma(s, xt):
            xtv = xt.rearrange("p (t x) -> p t x", t=SEG)
            for q in range(NHF):
                t0, t1 = s * SEG + q * 4, s * SEG + (q + 1) * 4
                tq = slice(q * 4, (q + 1) * 4)
                nc.sync.dma_start(
                    xtv[0:64, tq],
                    xs[t0:t1, :, 0:RC].rearrange("t c x -> c t x"))
                nc.sync.dma_start(
                    xtv[64:128, tq],
                    xs[t0:t1, :, WP:WP + RC].rearrange("t c x -> c t x"))

        xt_next = xpool.tile([128, SEG * RC], bf16, name="xt")
        emit_xt_dma(0, xt_next)
        nc.sync.dma_start(w_sb[:, :], wt[:, :])
        nc.sync.dma_start(cst_sb[:, :], cst[:, :])

        h_prev = None
        for s in range(NSEG):
            xt = xt_next
            xv = xt.rearrange("p (t r c) -> p t r c", t=SEG, r=HP, c=WP)

            SIGt = sigpool.tile([128, NF], bf16, name="SIGt")
            Ut = upool.tile([128, NF], bf16, name="Ut")
            Ht = hpool.tile([128, NF], bf16, name="Ht")
            Rt = rpool.tile([128, DNS], bf16, name="Rt")
            SYt = spool.tile([128, DNS], bf16, name="SYt")
            It = ipool.tile([128, DNS], bf16, name="It")
            sigv = SIGt.rearrange("p (x t) -> p x t", t=TS)
            uv = Ut.rearrange("p (x t) -> p x t", t=TS)

            # scan-restart column: f=0 at slot 0 of every pixel
            nc.gpsimd.memset(sigv[:, :, 0], 0.0)
            # u slot0 = h_{-1} for this segment (chains segments)
            if h_prev is None:
                nc.gpsimd.tensor_copy(uv[:, :, 0], g0c)
            else:
                hv = h_prev.rearrange("p (x t) -> p x t", t=TS)
                nc.gpsimd.tensor_copy(uv[:, :, 0], hv[:, :, SEG])
            for hf in range(NHF):
                ps = pspool.tile([128, 2048], f32, name="ps")
                for kappa in range(2):           # 0 = diff, 1 = h_tilde
                    po = kappa * 1024
                    for p in range(6):
                        blk = (kappa * 6 + p) * 64
                        lhsT = w_sb[:, blk:blk + 64]
                        r0, c0 = (0, p) if p < 3 else (2, p - 3)
                        for sp in range(2):
                            t0 = hf * 4 + sp * 2
                            for cs_ in range(2):
                                rhs = xv[0:128, t0:t0 + 2,
                                         r0 + 8 * cs_:r0 + 8 * cs_ + 8,
                                         c0:c0 + 32]
                                dst = ps[cs_ * 64:(cs_ + 1) * 64,
                                         po + sp * 512:po + sp * 512 + 512]
                                nc.tensor.matmul(
                                    dst, lhsT, rhs,
                                    start=(p == 0), stop=(p == 5),
                                    tile_position=(0, cs_ * 64))

                sl = slice(hf * 1024, (hf + 1) * 1024)
                pdq = ps[:, 0:1024].rearrange("p (a s x) -> p a x s",
                                              a=2, s=2, x=PXH)
                phq = ps[:, 1024:2048].rearrange("p (a s x) -> p a x s",
                                                 a=2, s=2, x=PXH)
                # psum eviction - all on ACT:
                # f = sigmoid(diff + bd) -> scan slots (runs of 2)
                sdst = sigv[:, :, 1 + hf * 4:5 + hf * 4].rearrange(
                    "p x (a s) -> p a x s", a=2, s=2)
                nc.scalar.activation(sdst, pdq, AF.Sigmoid, bias=bd2)
                # i = sigmoid(-diff - bd) -> dense (sp, px, st) layout
                nc.scalar.activation(
                    It[:, sl], pdq, AF.Sigmoid, bias=nbd2, scale=-1.0)
                # sigy = sigmoid(ht + bh); r = relu(ht + bh)
                nc.scalar.activation(SYt[:, sl], phq, AF.Sigmoid, bias=bh2)
                nc.scalar.activation(Rt[:, sl], phq, AF.Relu, bias=bh2)
                # s = min(sigy, 0.5) in place; g = s + r; u = i * g (DVE)
                nc.vector.tensor_scalar(
                    SYt[:, sl], SYt[:, sl], 0.5, None, OP.min)
                nc.vector.tensor_tensor(
                    Rt[:, sl], SYt[:, sl], Rt[:, sl], OP.add)
                udst = uv[:, :, 1 + hf * 4:5 + hf * 4].rearrange(
                    "p x (a s) -> p a x s", a=2, s=2)
                ivq = It[:, sl].rearrange("p (a x s) -> p a x s",
                                          a=2, x=PXH, s=2)
                gvq = Rt[:, sl].rearrange("p (a x s) -> p a x s",
                                          a=2, x=PXH, s=2)
                nc.vector.tensor_tensor(udst, ivq, gvq, OP.mult)

            # prefetch next segment's input before this segment's output
            # DMAs (sync queue is FIFO)
            if s + 1 < NSEG:
                xt_next = xpool.tile([128, SEG * RC], bf16, name="xt")
                emit_xt_dma(s + 1, xt_next)

            # h = scan: state = f * state + u, per-pixel chains
            nch = 4 if s == NSEG - 1 else NCH
            for k in range(nch):
                cw = PXH // nch * TS
                a, b2 = k * cw, (k + 1) * cw
                nc.vector.tensor_tensor_scan(
                    Ht[:, a:b2], SIGt[:, a:b2], Ut[:, a:b2],
                    0.0, OP.mult, OP.add)
                nc.sync.dma_start(out[s, :, a:b2], Ht[:, a:b2])
            h_prev = Ht
    nc.finalize()
    return nc


def _g0(h0):
    return np.where(h0 >= 0.0, h0 + 0.5, 1.0 / (1.0 + np.exp(-h0))).astype(np.float32)


def kernel(x, conv_w, conv_b, h0):
    x = np.asarray(x, np.float32)
    conv_w = np.asarray(conv_w, np.float32)
    conv_b = np.asarray(conv_b, np.float32)
    h0 = np.asarray(h0, np.float32)

    if "nc" not in _CACHE:
        _CACHE["nc"] = _build()
    nc = _CACHE["nc"]

    wd = conv_w[0:64] - conv_w[64:128]           # [64out, 64in, 3, 3]
    wh = conv_w[128:192]
    bd = conv_b[0:64] - conv_b[64:128]
    bh = conv_b[128:192]

    wt = np.zeros((128, 768), np.float32)
    for kappa, wk in ((0, wd), (1, wh)):
        for p in range(6):
            blk = (kappa * 6 + p) * 64
            if p < 3:
                # paired taps (0,p) on rows 0:64, (1,p) on rows 64:128
                wt[0:64, blk:blk + 64] = wk[:, :, 0, p].T
                wt[64:128, blk:blk + 64] = wk[:, :, 1, p].T
            else:
                # single tap (2, p-3); rows 64:128 stay zero
                wt[0:64, blk:blk + 64] = wk[:, :, 2, p - 3].T
    wt = wt.astype(BF16)

    x4 = x.reshape(B, T, C, H, W)
    g0f = _g0(h0)                                 # [B, C, H, W]

    bd2 = np.concatenate([bd, bd])[:, None]
    bh2 = np.concatenate([bh, bh])[:, None]

    in_maps = []
    for c in range(8):
        b, half = c // 2, c % 2
        xsh = np.zeros((T, C, HP + 1, WP), np.float32)
        if half == 0:
            xsh[:, :, 1:18, 1:33] = x4[b, :, :, 0:17, :]
        else:
            xsh[:, :, 0:17, 1:33] = x4[b, :, :, 15:32, :]
        xsh = xsh.reshape(T, C, RCE).astype(BF16)
        r16 = g0f[b, :, 16 * half:16 * half + 16, :]     # [64, 16, 32]
        g0c = np.concatenate(
            [r16[:, 0:8, :].reshape(64, PXH),
             r16[:, 8:16, :].reshape(64, PXH)], 0)       # [128, 256]
        cstc = np.concatenate(
            [bd2, bh2, -bd2, g0c], 1).astype(np.float32)
        in_maps.append({"xs": xsh, "wt": wt, "cst": cstc})

    _CACHE["in_maps"] = in_maps
    res = run_bass_kernel_spmd(nc, in_maps, core_ids=list(range(8)))

    outf = np.empty((B, T, C, H, W), np.float32)
    for c in range(8):
        b, half = c // 2, c % 2
        arr = np.asarray(res.results[c]["out"], np.float32)
        arr = arr.reshape(NSEG, 128, PXH, TS)[:, :, :, 1:]
        arr = arr.transpose(0, 3, 1, 2).reshape(T, 128, 8, 32)
        outf[b, :, :, 16 * half:16 * half + 8, :] = arr[:, 0:64]
        outf[b, :, :, 16 * half + 8:16 * half + 16, :] = arr[:, 64:128]
    return outf.reshape(B * T, C, H, W)


# revision 17
# speedup vs baseline: 1.2407x; 1.1959x over previous

# Trainium2 Bass kernel for MinConvExpLSTMCell (v12).
#
# Math (linear-space reformulation of the reference's log-space scan):
#   y = conv3x3(x, W) + b; [f_gate, i_gate, h_tilde] = split(y)
#   diff = f_gate - i_gate = conv(x, W_f - W_i) + (b_f - b_i)
#   f = sigmoid(diff);  i = sigmoid(-diff)  (both on ACT, full precision)
#   g = min(sigmoid(y), 0.5) + relu(y),  y = h_tilde + b_h
#     (sigmoid(min(y,0)) == min(sigmoid(y),0.5) by monotonicity - exact)
#   h_t = f_t * h_{t-1} + i_t * g_t,  h_{-1} = g(h0)
#
# Sharding: 8 cores = 4 batches x 2 spatial halves (16 output rows each).
#
# Matmul: K=128 tap-pair packing - x stored twice in SBUF (partitions
# 0:63 "copy A", partitions 64:127 shifted down one image row "copy B"),
# one K=128 matmul contracts two vertically-adjacent taps at once.
# 2x column tiling gives pixel-split psum (partitions 0:63 = rows 0:8 of
# the half-image, 64:127 = rows 8:16) so post runs on 128 partitions.
# rhs free dims (steps:2, rows:8, cols:32) keep the stream contiguous;
# evacuation iterates (sp, px, st) so scan-slot writes are runs-of-2.
#
# Engines per 4-step group: ACT = f/i/sigy sigmoids + relu (all psum
# eviction); DVE = s=min(sigy,.5), g=s+r, scan; GpSimd = u=i*g, memset,
# chain copy (last segment's u on DVE, 4 scan chunks, to cut the tail).
# Input DMAs are emitted before output DMAs (FIFO queues).

import sys
import numpy as np

sys.path.insert(0, "/opt/trn_rl_repo")

import ml_dtypes
from contextlib import ExitStack

import concourse.bass as bass
import concourse.bacc as bacc
import concourse.mybir as mybir
from concourse.tile import TileContext
from concourse.bass_utils import run_bass_kernel_spmd

BF16 = ml_dtypes.bfloat16
B, T, C, H, W = 4, 64, 64, 32, 32
SEG = 8
NSEG = T // SEG            # 8
NHF = SEG // 4             # 2 four-step groups per segment
HP, WP = 18, 34            # padded shard rows/cols
RC = HP * WP               # 612
RCE = RC + WP              # 646: one extra zero row for shifted copy B
PXH = 256                  # pixels per column-strip (8 rows x 32 cols)
TS = SEG + 1               # scan slots per pixel per segment
NF = PXH * TS              # 2304 scan free size
DNS = SEG * PXH            # 2048 dense free size
NCH = 2                    # scan pixel-chunks per segment
CPX = PXH // NCH           # 128 pixels per chunk

_CACHE = {}


def _build():
    f32 = mybir.dt.float32
    bf16 = mybir.dt.bfloat16
    AF = mybir.ActivationFunctionType
    OP = mybir.AluOpType

    nc = bacc.Bacc()
    xs = nc.dram_tensor("xs", [T, C, RCE], bf16, kind="ExternalInput")
    wt = nc.dram_tensor("wt", [128, 768], bf16, kind="ExternalInput")
    cst = nc.dram_tensor("cst", [128, 3 + PXH], f32, kind="ExternalInput")
    out = nc.dram_tensor("out", [NSEG, 128, NF], bf16, kind="ExternalOutput")

    with TileContext(nc) as tc, ExitStack() as ctx:
        cpool = ctx.enter_context(tc.tile_pool(name="consts", bufs=1))
        xpool = ctx.enter_context(tc.tile_pool(name="x", bufs=3))
        pspool = ctx.enter_context(tc.tile_pool(name="ps", bufs=2, space="PSUM"))
        sigpool = ctx.enter_context(tc.tile_pool(name="sig", bufs=3))
        rpool = ctx.enter_context(tc.tile_pool(name="r", bufs=2))
        spool = ctx.enter_context(tc.tile_pool(name="s", bufs=2))
        ipool = ctx.enter_context(tc.tile_pool(name="i", bufs=2))
        upool = ctx.enter_context(tc.tile_pool(name="u", bufs=3))
        hpool = ctx.enter_context(tc.tile_pool(name="h", bufs=3))

        w_sb = cpool.tile([128, 768], bf16)
        nc.sync.dma_start(w_sb[:, :], wt[:, :])
        cst_sb = cpool.tile([128, 3 + PXH], f32)
        nc.sync.dma_start(cst_sb[:, :], cst[:, :])
        bd2 = cst_sb[:, 0:1]
        bh2 = cst_sb[:, 1:2]
        nbd2 = cst_sb[:, 2:3]
        g0c = cst_sb[:, 3:3 + PXH]

        def emit_xt_dma(s, xt):
            xtv = xt.rearrange("p (t x) -> p t x", t=SEG)
            for q in range(NHF):
                t0, t1 = s * SEG + q * 4, s * SEG + (q + 1) * 4
                tq = slice(q * 4, (q + 1) * 4)
                nc.sync.dma_start(
                    xtv[0:64, tq],
                    xs[t0:t1, :, 0:RC].rearrange("t c x -> c t x"))
                nc.sync.dma_start(
                    xtv[64:128, tq],
                    xs[t0:t1, :, WP:WP + RC].rearrange("t c x -> c t x"))

        xt_next = xpool.tile([128, SEG * RC], bf16, name="xt")
        emit_xt_dma(0, xt_next)

        h_prev = None
        for s in range(NSEG):
            xt = xt_next
            xv = xt.rearrange("p (t r c) -> p t r c", t=SEG, r=HP, c=WP)

            SIGt = sigpool.tile([128, NF], bf16, name="SIGt")
            Ut = upool.tile([128, NF], bf16, name="Ut")
            Ht = hpool.tile([128, NF], bf16, name="Ht")
            Rt = rpool.tile([128, DNS], bf16, name="Rt")
            SYt = spool.tile([128, DNS], bf16, name="SYt")
            It = ipool.tile([128, DNS], bf16, name="It")
            sigv = SIGt.rearrange("p (x t) -> p x t", t=TS)
            uv = Ut.rearrange("p (x t) -> p x t", t=TS)

            # scan-restart column: f=0 at slot 0 of every pixel
            nc.gpsimd.memset(sigv[:, :, 0], 0.0)
            # u slot0 = h_{-1} for this segment (chains segments)
            if h_prev is None:
                nc.gpsimd.tensor_copy(uv[:, :, 0], g0c)
            else:
                hv = h_prev.rearrange("p (x t) -> p x t", t=TS)
                nc.gpsimd.tensor_copy(uv[:, :, 0], hv[:, :, SEG])
            for hf in range(NHF):
                ps = pspool.tile([128, 2048], f32, name="ps")
                for kappa in range(2):           # 0 = diff, 1 = h_tilde
                    po = kappa * 1024
                    for p in range(6):
                        blk = (kappa * 6 + p) * 64
                        lhsT = w_sb[:, blk:blk + 64]
                        r0, c0 = (0, p) if p < 3 else (2, p - 3)
                        for sp in range(2):
                            t0 = hf * 4 + sp * 2
                            for cs_ in range(2):
                                rhs = xv[0:128, t0:t0 + 2,
                                         r0 + 8 * cs_:r0 + 8 * cs_ + 8,
                                         c0:c0 + 32]
                                dst = ps[cs_ * 64:(cs_ + 1) * 64,
                                         po + sp * 512:po + sp * 512 + 512]
                                nc.tensor.matmul(
                                    dst, lhsT, rhs,
                                    start=(p == 0), stop=(p == 5),
                                    tile_position=(0, cs_ * 64))

                sl = slice(hf * 1024, (hf + 1) * 1024)
                pdq = ps[:, 0:1024].rearrange("p (a s x) -> p a x s",
                                              a=2, s=2, x=PXH)
                phq = ps[:, 1024:2048].rearrange("p (a s x) -> p a x s",
                                                 a=2, s=2, x=PXH)
                # psum eviction - all on ACT:
                # f = sigmoid(diff + bd) -> scan slots (runs of 2)
                sdst = sigv[:, :, 1 + hf * 4:5 + hf * 4].rearrange(
                    "p x (a s) -> p a x s", a=2, s=2)
                nc.scalar.activation(sdst, pdq, AF.Sigmoid, bias=bd2)
                # i = sigmoid(-diff - bd) -> dense (sp, px, st) layout
                nc.scalar.activation(
                    It[:, sl], pdq, AF.Sigmoid, bias=nbd2, scale=-1.0)
                # sigy = sigmoid(ht + bh); r = relu(ht + bh)
                nc.scalar.activation(SYt[:, sl], phq, AF.Sigmoid, bias=bh2)
                nc.scalar.activation(Rt[:, sl], phq, AF.Relu, bias=bh2)
                # s = min(sigy, 0.5) in place; g = s + r; u = i * g (DVE)
                nc.vector.tensor_scalar(
                    SYt[:, sl], SYt[:, sl], 0.5, None, OP.min)
                nc.vector.tensor_tensor(
                    Rt[:, sl], SYt[:, sl], Rt[:, sl], OP.add)
                udst = uv[:, :, 1 + hf * 4:5 + hf * 4].rearrange(
                    "p x (a s) -> p a x s", a=2, s=2)
                ivq = It[:, sl].rearrange("p (a x s) -> p a x s",
                                          a=2, x=PXH, s=2)
                gvq = Rt[:, sl].rearrange("p (a x s) -> p a x s",
                                          a=2, x=PXH, s=2)
                nc.vector.tensor_tensor(udst, ivq, gvq, OP.mult)

            # prefetch next segment's input before this segment's output
            # DMAs (sync queue is FIFO)
            if s + 1 < NSEG:
                xt_next = xpool.tile([128, SEG * RC], bf16, name="xt")
                emit_xt_dma(s + 1, xt_next)

            # h = scan: state = f * state + u, per-pixel chains
            for k in range(NCH):
                a, b2 = k * CPX * TS, (k + 1) * CPX * TS
                nc.vector.tensor_tensor_scan(
                    Ht[:, a:b2], SIGt[:, a:b2], Ut[:, a:b2],
                    0.0, OP.mult, OP.add)
                nc.sync.dma_start(out[s, :, a:b2], Ht[:, a:b2])
            h_prev = Ht
    nc.finalize()
    return nc


def _g0(h0):
    return np.where(h0 >= 0.0, h0 + 0.5, 1.0 / (1.0 + np.exp(-h0))).astype(np.float32)


def kernel(x, conv_w, conv_b, h0):
    x = np.asarray(x, np.float32)
    conv_w = np.asarray(conv_w, np.float32)
    conv_b = np.asarray(conv_b, np.float32)
    h0 = np.asarray(h0, np.float32)

    if "nc" not in _CACHE:
        _CACHE["nc"] = _build()
    nc = _CACHE["nc"]

    wd = conv_w[0:64] - conv_w[64:128]           # [64out, 64in, 3, 3]
    wh = conv_w[128:192]
    bd = conv_b[0:64] - conv_b[64:128]
    bh = conv_b[128:192]

    wt = np.zeros((128, 768), np.float32)
    for kappa, wk in ((0, wd), (1, wh)):
        for p in range(6):
            blk = (kappa * 6 + p) * 64
            if p < 3:
                # paired taps (0,p) on rows 0:64, (1,p) on rows 64:128
                wt[0:64, blk:blk + 64] = wk[:, :, 0, p].T
                wt[64:128, blk:blk + 64] = wk[:, :, 1, p].T
            else:
                # single tap (2, p-3); rows 64:128 stay zero
                wt[0:64, blk:blk + 64] = wk[:, :, 2, p - 3].T
    wt = wt.astype(BF16)

    x4 = x.reshape(B, T, C, H, W)
    g0f = _g0(h0)                                 # [B, C, H, W]

    bd2 = np.concatenate([bd, bd])[:, None]
    bh2 = np.concatenate([bh, bh])[:, None]

    in_maps = []
    for c in range(8):
        b, half = c // 2, c % 2
        xsh = np.zeros((T, C, HP + 1, WP), np.float32)
        if half == 0:
            xsh[:, :, 1:18, 1:33] = x4[b, :, :, 0:17, :]
        else:
            xsh[:, :, 0:17, 1:33] = x4[b, :, :, 15:32, :]
        xsh = xsh.reshape(T, C, RCE).astype(BF16)
        r16 = g0f[b, :, 16 * half:16 * half + 16, :]     # [64, 16, 32]
        g0c = np.concatenate(
            [r16[:, 0:8, :].reshape(64, PXH),
             r16[:, 8:16, :].reshape(64, PXH)], 0)       # [128, 256]
        cstc = np.concatenate(
            [bd2, bh2, -bd2, g0c], 1).astype(np.float32)
        in_maps.append({"xs": xsh, "wt": wt, "cst": cstc})

    _CACHE["in_maps"] = in_maps
    res = run_bass_kernel_spmd(nc, in_maps, core_ids=list(range(8)))

    outf = np.empty((B, T, C, H, W), np.float32)
    for c in range(8):
        b, half = c // 2, c % 2
        arr = np.asarray(res.results[c]["out"], np.float32)
        arr = arr.reshape(NSEG, 128, PXH, TS)[:, :, :, 1:]
        arr = arr.transpose(0, 3, 1, 2).reshape(T, 128, 8, 32)
        outf[b, :, :, 16 * half:16 * half + 8, :] = arr[:, 0:64]
        outf[b, :, :, 16 * half + 8:16 * half + 16, :] = arr[:, 64:128]
    return outf.reshape(B * T, C, H, W)


# revision 19
# speedup vs baseline: 1.2430x; 1.0019x over previous

# Trainium2 Bass kernel for MinConvExpLSTMCell (final: 129.6us HW, rel err 9.2e-3).
#
# Math (linear-space reformulation of the reference's log-space scan):
#   y = conv3x3(x, W) + b; [f_gate, i_gate, h_tilde] = split(y)
#   diff = f_gate - i_gate = conv(x, W_f - W_i) + (b_f - b_i)
#   f = sigmoid(diff);  i = sigmoid(-diff)  (both on ACT, full precision)
#   g = min(sigmoid(y), 0.5) + relu(y),  y = h_tilde + b_h
#     (sigmoid(min(y,0)) == min(sigmoid(y),0.5) by monotonicity - exact)
#   h_t = f_t * h_{t-1} + i_t * g_t,  h_{-1} = g(h0)
#
# Sharding: 8 cores = 4 batches x 2 spatial halves (16 output rows each).
#
# Matmul: K=128 tap-pair packing - x stored twice in SBUF (partitions
# 0:63 "copy A", partitions 64:127 shifted down one image row "copy B"),
# one K=128 matmul contracts two vertically-adjacent taps at once.
# 2x column tiling gives pixel-split psum (partitions 0:63 = rows 0:8 of
# the half-image, 64:127 = rows 8:16) so post runs on 128 partitions.
# rhs free dims (steps:2, rows:8, cols:32) keep the stream contiguous;
# evacuation iterates (sp, px, st) so scan-slot writes are runs-of-2.
#
# Engines per 4-step group: ACT = f/i/sigy sigmoids + relu (all psum
# eviction); DVE = s=min(sigy,.5), g=s+r, u=i*g, scan; GpSimd = memset,
# chain copy. Input DMAs are emitted before output DMAs (FIFO queues).

import sys
import numpy as np

sys.path.insert(0, "/opt/trn_rl_repo")

import ml_dtypes
from contextlib import ExitStack

import concourse.bass as bass
import concourse.bacc as bacc
import concourse.mybir as mybir
from concourse.tile import TileContext
from concourse.bass_utils import run_bass_kernel_spmd

BF16 = ml_dtypes.bfloat16
B, T, C, H, W = 4, 64, 64, 32, 32
SEG = 8
NSEG = T // SEG            # 8
NHF = SEG // 4             # 2 four-step groups per segment
HP, WP = 18, 34            # padded shard rows/cols
RC = HP * WP               # 612
RCE = RC + WP              # 646: one extra zero row for shifted copy B
PXH = 256                  # pixels per column-strip (8 rows x 32 cols)
TS = SEG + 1               # scan slots per pixel per segment
NF = PXH * TS              # 2304 scan free size
DNS = SEG * PXH            # 2048 dense free size
NCH = 2                    # scan pixel-chunks per segment
CPX = PXH // NCH           # 128 pixels per chunk

_CACHE = {}


def _build():
    f32 = mybir.dt.float32
    bf16 = mybir.dt.bfloat16
    AF = mybir.ActivationFunctionType
    OP = mybir.AluOpType

    nc = bacc.Bacc()
    xs = nc.dram_tensor("xs", [T, C, RCE], bf16, kind="ExternalInput")
    wt = nc.dram_tensor("wt", [128, 768], bf16, kind="ExternalInput")
    cst = nc.dram_tensor("cst", [128, 3 + PXH], f32, kind="ExternalInput")
    out = nc.dram_tensor("out", [NSEG, 128, NF], bf16, kind="ExternalOutput")

    with TileContext(nc) as tc, ExitStack() as ctx:
        cpool = ctx.enter_context(tc.tile_pool(name="consts", bufs=1))
        xpool = ctx.enter_context(tc.tile_pool(name="x", bufs=3))
        pspool = ctx.enter_context(tc.tile_pool(name="ps", bufs=2, space="PSUM"))
        sigpool = ctx.enter_context(tc.tile_pool(name="sig", bufs=3))
        rpool = ctx.enter_context(tc.tile_pool(name="r", bufs=2))
        spool = ctx.enter_context(tc.tile_pool(name="s", bufs=2))
        ipool = ctx.enter_context(tc.tile_pool(name="i", bufs=2))
        upool = ctx.enter_context(tc.tile_pool(name="u", bufs=3))
        hpool = ctx.enter_context(tc.tile_pool(name="h", bufs=3))

        w_sb = cpool.tile([128, 768], bf16)
        nc.sync.dma_start(w_sb[:, :], wt[:, :])
        cst_sb = cpool.tile([128, 3 + PXH], f32)
        bd2 = cst_sb[:, 0:1]
        bh2 = cst_sb[:, 1:2]
        nbd2 = cst_sb[:, 2:3]
        g0c = cst_sb[:, 3:3 + PXH]

        def emit_xt_dma(s, xt, quarters=range(NHF)):
            xtv = xt.rearrange("p (t x) -> p t x", t=SEG)
            for q in quarters:
                t0, t1 = s * SEG + q * 4, s * SEG + (q + 1) * 4
                tq = slice(q * 4, (q + 1) * 4)
                nc.sync.dma_start(
                    xtv[0:64, tq],
                    xs[t0:t1, :, 0:RC].rearrange("t c x -> c t x"))
                nc.sync.dma_start(
                    xtv[64:128, tq],
                    xs[t0:t1, :, WP:WP + RC].rearrange("t c x -> c t x"))

        xt_next = xpool.tile([128, SEG * RC], bf16, name="xt")
        emit_xt_dma(0, xt_next, [0])
        nc.sync.dma_start(cst_sb[:, :], cst[:, :])
        emit_xt_dma(0, xt_next, [1])

        h_prev = None
        for s in range(NSEG):
            xt = xt_next
            xv = xt.rearrange("p (t r c) -> p t r c", t=SEG, r=HP, c=WP)

            SIGt = sigpool.tile([128, NF], bf16, name="SIGt")
            Ut = upool.tile([128, NF], bf16, name="Ut")
            Ht = hpool.tile([128, NF], bf16, name="Ht")
            Rt = rpool.tile([128, DNS], bf16, name="Rt")
            SYt = spool.tile([128, DNS], bf16, name="SYt")
            It = ipool.tile([128, DNS], bf16, name="It")
            sigv = SIGt.rearrange("p (x t) -> p x t", t=TS)
            uv = Ut.rearrange("p (x t) -> p x t", t=TS)

            # scan-restart column: f=0 at slot 0 of every pixel
            nc.gpsimd.memset(sigv[:, :, 0], 0.0)
            # u slot0 = h_{-1} for this segment (chains segments)
            if h_prev is None:
                nc.gpsimd.tensor_copy(uv[:, :, 0], g0c)
            else:
                hv = h_prev.rearrange("p (x t) -> p x t", t=TS)
                nc.gpsimd.tensor_copy(uv[:, :, 0], hv[:, :, SEG])
            for hf in range(NHF):
                ps = pspool.tile([128, 2048], f32, name="ps")
                for kappa in range(2):           # 0 = diff, 1 = h_tilde
                    po = kappa * 1024
                    for p in range(6):
                        blk = (kappa * 6 + p) * 64
                        lhsT = w_sb[:, blk:blk + 64]
                        r0, c0 = (0, p) if p < 3 else (2, p - 3)
                        for sp in range(2):
                            t0 = hf * 4 + sp * 2
                            for cs_ in range(2):
                                rhs = xv[0:128, t0:t0 + 2,
                                         r0 + 8 * cs_:r0 + 8 * cs_ + 8,
                                         c0:c0 + 32]
                                dst = ps[cs_ * 64:(cs_ + 1) * 64,
                                         po + sp * 512:po + sp * 512 + 512]
                                nc.tensor.matmul(
                                    dst, lhsT, rhs,
                                    start=(p == 0), stop=(p == 5),
                                    tile_position=(0, cs_ * 64))

                sl = slice(hf * 1024, (hf + 1) * 1024)
                pdq = ps[:, 0:1024].rearrange("p (a s x) -> p a x s",
                                              a=2, s=2, x=PXH)
                phq = ps[:, 1024:2048].rearrange("p (a s x) -> p a x s",
                                                 a=2, s=2, x=PXH)
                # psum eviction + post; the very last group is split
                # into pixel-halves so ACT/DVE/scan pipeline in the tail
                last = (s == NSEG - 1 and hf == NHF - 1)
                sdst4 = sigv[:, :, 1 + hf * 4:5 + hf * 4].rearrange(
                    "p x (a s) -> p a x s", a=2, s=2)
                udst4 = uv[:, :, 1 + hf * 4:5 + hf * 4].rearrange(
                    "p x (a s) -> p a x s", a=2, s=2)
                iv4 = It[:, sl].rearrange("p (a x s) -> p a x s",
                                          a=2, x=PXH, s=2)
                sy4 = SYt[:, sl].rearrange("p (a x s) -> p a x s",
                                           a=2, x=PXH, s=2)
                rt4 = Rt[:, sl].rearrange("p (a x s) -> p a x s",
                                          a=2, x=PXH, s=2)
                for m in ([0, 1] if last else [None]):
                    px = slice(None) if m is None else slice(m * 128, (m + 1) * 128)
                    sd, ud = sdst4[:, :, px], udst4[:, :, px]
                    pd, ph = pdq[:, :, px], phq[:, :, px]
                    iq, syq, rq = iv4[:, :, px], sy4[:, :, px], rt4[:, :, px]
                    nc.scalar.activation(sd, pd, AF.Sigmoid, bias=bd2)
                    nc.scalar.activation(iq, pd, AF.Sigmoid, bias=nbd2,
                                         scale=-1.0)
                    nc.scalar.activation(syq, ph, AF.Sigmoid, bias=bh2)
                    nc.scalar.activation(rq, ph, AF.Relu, bias=bh2)
                    nc.vector.tensor_scalar(syq, syq, 0.5, None, OP.min)
                    nc.vector.tensor_tensor(rq, syq, rq, OP.add)
                    nc.vector.tensor_tensor(ud, iq, rq, OP.mult)

            # prefetch next segment's input before this segment's output
            # DMAs (sync queue is FIFO)
            if s + 1 < NSEG:
                xt_next = xpool.tile([128, SEG * RC], bf16, name="xt")
                emit_xt_dma(s + 1, xt_next)

            # h = scan: state = f * state + u, per-pixel chains
            for k in range(NCH):
                a, b2 = k * CPX * TS, (k + 1) * CPX * TS
                nc.vector.tensor_tensor_scan(
                    Ht[:, a:b2], SIGt[:, a:b2], Ut[:, a:b2],
                    0.0, OP.mult, OP.add)
                nc.sync.dma_start(out[s, :, a:b2], Ht[:, a:b2])
            h_prev = Ht
    nc.finalize()
    return nc


def _g0(h0):
    return np.where(h0 >= 0.0, h0 + 0.5, 1.0 / (1.0 + np.exp(-h0))).astype(np.float32)


def kernel(x, conv_w, conv_b, h0):
    x = np.asarray(x, np.float32)
    conv_w = np.asarray(conv_w, np.float32)
    conv_b = np.asarray(conv_b, np.float32)
    h0 = np.asarray(h0, np.float32)

    if "nc" not in _CACHE:
        _CACHE["nc"] = _build()
    nc = _CACHE["nc"]

    wd = conv_w[0:64] - conv_w[64:128]           # [64out, 64in, 3, 3]
    wh = conv_w[128:192]
    bd = conv_b[0:64] - conv_b[64:128]
    bh = conv_b[128:192]

    wt = np.zeros((128, 768), np.float32)
    for kappa, wk in ((0, wd), (1, wh)):
        for p in range(6):
            blk = (kappa * 6 + p) * 64
            if p < 3:
                # paired taps (0,p) on rows 0:64, (1,p) on rows 64:128
                wt[0:64, blk:blk + 64] = wk[:, :, 0, p].T
                wt[64:128, blk:blk + 64] = wk[:, :, 1, p].T
            else:
                # single tap (2, p-3); rows 64:128 stay zero
                wt[0:64, blk:blk + 64] = wk[:, :, 2, p - 3].T
    wt = wt.astype(BF16)

    x4 = x.reshape(B, T, C, H, W)
    g0f = _g0(h0)                                 # [B, C, H, W]

    bd2 = np.concatenate([bd, bd])[:, None]
    bh2 = np.concatenate([bh, bh])[:, None]

    in_maps = []
    for c in range(8):
        b, half = c // 2, c % 2
        xsh = np.zeros((T, C, HP + 1, WP), np.float32)
        if half == 0:
            xsh[:, :, 1:18, 1:33] = x4[b, :, :, 0:17, :]
        else:
            xsh[:, :, 0:17, 1:33] = x4[b, :, :, 15:32, :]
        xsh = xsh.reshape(T, C, RCE).astype(BF16)
        r16 = g0f[b, :, 16 * half:16 * half + 16, :]     # [64, 16, 32]
        g0c = np.concatenate(
            [r16[:, 0:8, :].reshape(64, PXH),
             r16[:, 8:16, :].reshape(64, PXH)], 0)       # [128, 256]
        cstc = np.concatenate(
            [bd2, bh2, -bd2, g0c], 1).astype(np.float32)
        in_maps.append({"xs": xsh, "wt": wt, "cst": cstc})

    _CACHE["in_maps"] = in_maps
    res = run_bass_kernel_spmd(nc, in_maps, core_ids=list(range(8)))

    outf = np.empty((B, T, C, H, W), np.float32)
    for c in range(8):
        b, half = c // 2, c % 2
        arr = np.asarray(res.results[c]["out"], np.float32)
        arr = arr.reshape(NSEG, 128, PXH, TS)[:, :, :, 1:]
        arr = arr.transpose(0, 3, 1, 2).reshape(T, 128, 8, 32)
        outf[b, :, :, 16 * half:16 * half + 8, :] = arr[:, 0:64]
        outf[b, :, :, 16 * half + 8:16 * half + 16, :] = arr[:, 64:128]
    return outf.reshape(B * T, C, H, W)
